# revision 14
# baseline (speedup 1.0000x reference)
"""BIDAF attention-flow kernel for Trainium2 (Bass/Tile), 8-core data-parallel.

Reference computation (per batch b):
    S[t,j]  = H[t]·w_h + U[j]·w_u + sum_d H[t,d]*U[j,d]*w_hu[d]
    A       = softmax_j(S);          C2Q = A @ U
    b_att   = softmax_t(max_j S);    Q2C = b_att @ H   (broadcast over t)
    G       = [H, C2Q, H*C2Q, H*Q2C]        # [T, 4D]

Kernel strategy (per core, 8 batches):
  * Matmul moving/stationary operands are bf16 where it matters (the PE
    streams bf16 at 1 cycle/row vs ~4 for fp32); PSUM stays f32.
  * H arrives in TWO forms from the host: d-major bf16 HTb [dp,kc,t]
    (similarity matmul needs no on-chip transposes) and t-major *f32*
    Hf [p,c,d|1].  f32 matters for the elementwise G-blocks: DVE runs
    pure-f32 tensor_tensor at ~1.05 ns/col vs ~2.5-3.3 for bf16/mixed.
  * The Q2C row-reduction runs as an fp32 matmul over Hf (its wq column
    is stride-0-broadcast to a [128,128] stationary, so every PSUM row
    IS the broadcast Q2C - no separate ones-matmul broadcast).
  * S is computed TRANSPOSED (ST[j,t]) so softmax-attention (C2Q) consumes
    P=exp(ST) directly as lhsT.  sh[t]=H·w_h cancels inside softmax_j, so
    P = exp(shu + su[j]) with su as a per-partition ACT bias; sh re-enters
    only in the tiny [128,8] b_att weights wq = max_j(P) * exp(sh).
  * Ones-columns appended to Hf and Ub host-side give l[t]=sum_j P and
    Wsum=sum_t wq for free inside the C2Q/Q2C matmuls.
  * max_j P needs a partition reduce: PE re-transposes P in [128,128]
    blocks into PSUM and DVE reduce_max handles 4 chunks per op.
  * Software-pipelined loads: batch b+2's loads issue (sync ring) BEFORE
    batch b's store (scalar ring), so loads never queue behind a 3MB
    store in DMA-ring FIFO order.
  * G block 0 (a verbatim copy of H) never touches the device: the host
    splices the original f32 H into the output during unshard.  The device
    emits [C2Q | H*C2Q | H*Q2C] as one [T,768] f32 block per batch.
  * Tile emits multi-wait instructions; TRN2 allows 1 wait/instruction, so
    the bacc rust passes run on the traced module before compile.
"""

import sys

sys.path.insert(0, "/opt/trn_rl_repo")

import ml_dtypes
import numpy as np

import concourse.bass as bass
import concourse.mybir as mybir
from concourse import tile

B, T, J, D = 64, 1024, 128, 256
NCORES = 8
BPC = B // NCORES  # batches per core
P = 128
NT = T // P  # 8 t-chunks per batch
DA = D + 1  # feature dim + ones column
HT_W = 2 * T  # 2048
IN_W = HT_W + DA  # bf16 blob: [HTb | Ub]
F32 = mybir.dt.float32
BF16 = mybir.dt.bfloat16
AF = mybir.ActivationFunctionType
ALU = mybir.AluOpType
AX = mybir.AxisListType
BF = ml_dtypes.bfloat16


def build_kernel(nc, bpc):
    HF = nc.declare_dram_parameter("hf", [bpc, P, NT, DA], F32, isOutput=False)
    IN = nc.declare_dram_parameter("inb", [bpc, P, IN_W], BF16, isOutput=False)
    whT_in = nc.declare_dram_parameter("whT", [P, 2, 2], BF16, isOutput=False)
    w2_in = nc.declare_dram_parameter("w2", [1, 2 * D], F32, isOutput=False)
    ident_in = nc.declare_dram_parameter("ident", [P, P], BF16, isOutput=False)
    ones_in = nc.declare_dram_parameter("ones1", [1, P], F32, isOutput=False)
    G = nc.declare_dram_parameter("G", [bpc, T, 3 * D], F32, isOutput=True)

    with tile.TileContext(nc) as tc:
        with (
            tc.tile_pool(name="const", bufs=1) as const_pool,
            tc.tile_pool(name="in", bufs=3) as in_pool,
            tc.tile_pool(name="hf", bufs=3) as hf_pool,
            tc.tile_pool(name="u", bufs=2) as u_pool,
            tc.tile_pool(name="p", bufs=2) as p_pool,
            tc.tile_pool(name="g", bufs=2) as g_pool,
            tc.tile_pool(name="sm", bufs=2) as sm_pool,
            tc.tile_pool(name="stps", bufs=1, space="PSUM") as st_ps,
            tc.tile_pool(name="ptps", bufs=2, space="PSUM") as pt_ps,
            tc.tile_pool(name="cqps", bufs=2, space="PSUM") as cq_ps,
            tc.tile_pool(name="smps", bufs=2, space="PSUM") as sm_ps,
        ):
            # ---- constants ----
            ident = const_pool.tile([P, P], BF16)
            nc.sync.dma_start(ident[:], ident_in[:])
            ones1 = const_pool.tile([1, P], F32)
            nc.sync.dma_start(ones1[:], ones_in[:])
            whT = const_pool.tile([P, 2, 2], BF16)
            nc.sync.dma_start(whT[:], whT_in[:])
            # broadcast [w_hu; w_u] across partitions via a K=1 ones-matmul
            w2_sb = const_pool.tile([1, 2 * D], F32)
            nc.sync.dma_start(w2_sb[:], w2_in[:])
            wps = sm_ps.tile([P, 2 * D], F32, tag="sm")
            nc.tensor.matmul(wps[:], ones1[:], w2_sb[:], start=True, stop=True)
            wb = const_pool.tile([P, 2 * D], BF16)
            nc.scalar.copy(wb[:], wps[:])
            whu_b = wb[:, 0:D]
            wu_b = wb[:, D : 2 * D]

            def ht(inb, kc, lo, hi):  # HTb slice: [P, hi-lo] of chunk kc
                return inb[:, kc * T + lo : kc * T + hi]

            inb_tiles = {}
            hf_tiles = {}

            def load(b):
                inb_tiles[b] = in_pool.tile([P, IN_W], BF16, name="inb")
                nc.sync.dma_start(inb_tiles[b][:], IN[b])
                hf_tiles[b] = hf_pool.tile([P, NT, DA], F32, name="hf")
                nc.sync.dma_start(hf_tiles[b][:], HF[b])

            load(0)
            if bpc > 1:
                load(1)

            for b in range(bpc):
                inb = inb_tiles.pop(b)
                Hf = hf_tiles.pop(b)
                Uo = inb[:, HT_W:IN_W]

                # ---- U-side prep (gpsimd: SBUF-only ops) ----
                Uw = u_pool.tile([P, D], BF16)
                nc.gpsimd.tensor_mul(Uw[:], Uo[:, 0:D], whu_b)
                scr = sm_pool.tile([P, D], F32)
                su = sm_pool.tile([P, 1], F32)
                nc.gpsimd.tensor_mul(scr[:], Uo[:, 0:D], wu_b)
                nc.vector.reduce_sum(su[:], scr[:], axis=AX.X)
                uwt_ps = sm_ps.tile([P, 2, P], BF16, tag="sm")
                for kc in range(2):
                    nc.tensor.transpose(
                        uwt_ps[:, kc, :], Uw[:, kc * P : (kc + 1) * P], ident[:]
                    )
                UwT = u_pool.tile([P, 2, P], BF16)
                nc.scalar.copy(UwT[:], uwt_ps[:])

                # ---- shT[t-chunk, c] = HT-chunk.T @ w_h column ----
                shT_ps = sm_ps.tile([P, NT, 2], F32, tag="sm")
                for c in range(NT):
                    for kc in range(2):
                        nc.tensor.matmul(
                            shT_ps[:, c, :],
                            ht(inb, kc, c * P, (c + 1) * P),
                            whT[:, kc, :],
                            start=(kc == 0),
                            stop=(kc == 1),
                        )
                esh = sm_pool.tile([P, NT], F32)
                nc.scalar.activation(esh[:], shT_ps[:, :, 0], AF.Exp)

                # ---- similarity matmul: ST[j, t] ----
                st = st_ps.tile([P, T], F32, tag="st")
                for th in range(2):
                    for kc in range(2):
                        nc.tensor.matmul(
                            st[:, th * 512 : (th + 1) * 512],
                            UwT[:, kc, :],
                            ht(inb, kc, th * 512, (th + 1) * 512),
                            start=(kc == 0),
                            stop=(kc == 1),
                        )

                # ---- P = exp(shu + su[j]) ----
                Pt = p_pool.tile([P, T], BF16)
                nc.scalar.activation(Pt[:], st[:], AF.Exp, bias=su[:], scale=1.0)

                # ---- C2Q per t-chunk (starts as soon as exp is done) ----
                Gt = g_pool.tile([P, NT, 3 * D], F32)
                linv = sm_pool.tile([P, NT], F32)
                for c in range(NT):
                    cq = cq_ps.tile([P, DA], F32, tag="cq")
                    nc.tensor.matmul(
                        cq[:], Pt[:, c * P : (c + 1) * P], Uo[:], start=True,
                        stop=True,
                    )
                    nc.vector.reciprocal(linv[:, c : c + 1], cq[:, D : D + 1])
                    if c % 4 == 3:
                        nc.vector.tensor_scalar_mul(
                            Gt[:, c, 0:D], cq[:, 0:D], linv[:, c : c + 1]
                        )
                    else:
                        nc.scalar.activation(
                            Gt[:, c, 0:D], cq[:, 0:D], AF.Copy,
                            scale=linv[:, c : c + 1],
                        )

                # ---- G2 = H * C2Q, pure-f32, 4 chunks per op ----
                for h, eng in ((0, nc.gpsimd), (1, nc.vector)):
                    cs = slice(h * 4, h * 4 + 4)
                    eng.tensor_mul(
                        Gt[:, cs, D : 2 * D], Hf[:, cs, 0:D], Gt[:, cs, 0:D]
                    )

                # ---- max_j P via PE transpose + DVE reduce ----
                mx = sm_pool.tile([P, NT], F32)
                for h in range(2):
                    pt = pt_ps.tile([P, 4, P], BF16, tag="pt")
                    for i in range(4):
                        c = h * 4 + i
                        nc.tensor.transpose(
                            pt[:, i, :], Pt[:, c * P : (c + 1) * P], ident[:]
                        )
                    nc.vector.reduce_max(
                        mx[:, h * 4 : (h + 1) * 4].unsqueeze(2), pt[:], axis=AX.X
                    )
                wq = sm_pool.tile([P, NT], F32)
                nc.vector.tensor_mul(wq[:], mx[:], esh[:])

                # ---- Q2C: fp32 matmul over Hf; the wq column is stride-0
                # broadcast to a full [128,128] stationary so every PSUM row
                # IS the broadcast Q2C ----
                q2cu_ps = sm_ps.tile([P, DA], F32, tag="sm")
                for c in range(NT):
                    nc.tensor.matmul(
                        q2cu_ps[:],
                        wq[:, c : c + 1].broadcast_to((P, P)),
                        Hf[:, c, :],
                        start=(c == 0),
                        stop=(c == NT - 1),
                    )
                rinb = sm_pool.tile([P, 1], F32)
                nc.vector.reciprocal(rinb[:], q2cu_ps[:, D : D + 1])
                q2cb = sm_pool.tile([P, D], F32)
                nc.vector.tensor_scalar_mul(q2cb[:], q2cu_ps[:, 0:D], rinb[:])

                # ---- G3 = H * Q2C, pure-f32 per-chunk 2D ops ----
                for c in range(NT):
                    eng = nc.gpsimd if c % 2 == 0 else nc.vector
                    eng.tensor_mul(
                        Gt[:, c, 2 * D : 3 * D], Hf[:, c, 0:D], q2cb[:]
                    )

                # ---- prefetch batch b+2's inputs before this batch's store ----
                if b + 2 < bpc:
                    load(b + 2)

                # ---- store [C2Q | H*C2Q | H*Q2C] on the scalar ring ----
                Gb = G[b].rearrange("(c p) d -> p c d", p=P)
                nc.scalar.dma_start(Gb[:], Gt[:])

    return nc


_NC_CACHE = {}


def get_nc(bpc=BPC):
    key = bpc
    if key not in _NC_CACHE:
        import bass_rust as _bass_rust

        nc = bass.Bass()
        build_kernel(nc, bpc)
        # TRN2 allows at most 1 sync wait per instruction (2 on event
        # semaphores); Tile emits more.  These are the bacc lowering passes
        # that legalize the wait lists.
        _bass_rust.move_matmul_waits_to_ldweights(nc.m)
        _bass_rust.generate_event_semaphores(nc)
        # lower bass_isa subclasses (e.g. EVENT_SEMAPHORE_RANGE_CLEAR) into
        # raw InstISA encodings walrus can emit
        mybir.codegen_inst_isa_subclasses(nc)
        _NC_CACHE[key] = nc
    return _NC_CACHE[key]


def _prep_core(Hc, Uc):
    """Host-side packing for one core's batches."""
    bpc = Hc.shape[0]
    hf = np.ones((bpc, P, NT, DA), dtype=np.float32)
    hf[..., :D] = Hc.reshape(bpc, NT, P, D).transpose(0, 2, 1, 3)
    blob = np.empty((bpc, P, IN_W), dtype=BF)
    blob[:, :, :HT_W] = (
        Hc.reshape(bpc, T, 2, P).transpose(0, 3, 2, 1).reshape(bpc, P, HT_W)
    )
    ubv = blob[:, :, HT_W:]
    ubv[..., :D] = Uc
    ubv[..., D] = 1.0
    return hf, blob


def run(inputs, trace=False, **kwargs):
    from concourse.bass_utils import run_bass_kernel_spmd

    nc = get_nc(BPC)
    H = np.asarray(inputs["H"], dtype=np.float32)
    U = np.asarray(inputs["U"], dtype=np.float32)
    w_h = np.asarray(inputs["w_h"], dtype=np.float32)
    whT = np.ascontiguousarray(
        np.repeat(w_h.reshape(2, P).T[:, :, None], 2, axis=2)
    ).astype(BF)
    w2 = np.concatenate(
        [
            np.asarray(inputs["w_hu"], dtype=np.float32),
            np.asarray(inputs["w_u"], dtype=np.float32),
        ]
    ).reshape(1, 2 * D)
    ident = np.eye(P, dtype=BF)
    ones1 = np.ones((1, P), dtype=np.float32)
    in_maps = []
    for c in range(NCORES):
        hf, blob = _prep_core(
            H[c * BPC : (c + 1) * BPC], U[c * BPC : (c + 1) * BPC]
        )
        in_maps.append(
            {
                "hf": hf,
                "inb": blob,
                "whT": whT,
                "w2": w2,
                "ident": ident,
                "ones1": ones1,
            }
        )
    res = run_bass_kernel_spmd(
        nc, in_maps, core_ids=list(range(NCORES)), trace=trace, **kwargs
    )
    out = np.empty((B, T, 4 * D), dtype=np.float32)
    out[:, :, 0:D] = H  # G block 0 is a verbatim copy of H
    out[:, :, D:] = np.concatenate(
        [res.results[c]["G"] for c in range(NCORES)], axis=0
    )
    return out, res


def kernel(**inputs):
    out, _ = run(inputs, trace=False)
    return out


# revision 15
# speedup vs baseline: 1.0777x; 1.0777x over previous
"""BIDAF attention-flow kernel for Trainium2 (Bass/Tile), 8-core data-parallel.

Reference computation (per batch b):
    S[t,j]  = H[t]·w_h + U[j]·w_u + sum_d H[t,d]*U[j,d]*w_hu[d]
    A       = softmax_j(S);          C2Q = A @ U
    b_att   = softmax_t(max_j S);    Q2C = b_att @ H   (broadcast over t)
    G       = [H, C2Q, H*C2Q, H*Q2C]        # [T, 4D]

Kernel strategy (per core, 8 batches):
  * Matmul moving/stationary operands are bf16 where it matters (the PE
    streams bf16 at 1 cycle/row vs ~4 for fp32); PSUM stays f32.
  * H arrives in TWO forms from the host: d-major bf16 HTb [dp,kc,t]
    (similarity matmul needs no on-chip transposes) and t-major *f32*
    Hf [p,c,d|1].  f32 matters for the elementwise G-blocks: DVE runs
    pure-f32 tensor_tensor at ~1.05 ns/col vs ~2.5-3.3 for bf16/mixed.
  * The Q2C row-reduction runs as an fp32 matmul over Hf (its wq column
    is stride-0-broadcast to a [128,128] stationary, so every PSUM row
    IS the broadcast Q2C - no separate ones-matmul broadcast).
  * S is computed TRANSPOSED (ST[j,t]) so softmax-attention (C2Q) consumes
    P=exp(ST) directly as lhsT.  sh[t]=H·w_h cancels inside softmax_j, so
    P = exp(shu + su[j]) with su as a per-partition ACT bias; sh re-enters
    only in the tiny [128,8] b_att weights wq = max_j(P) * exp(sh).
  * Ones-columns appended to Hf and Ub host-side give l[t]=sum_j P and
    Wsum=sum_t wq for free inside the C2Q/Q2C matmuls.
  * max_j P needs a partition reduce: PE re-transposes P in [128,128]
    blocks into PSUM and DVE reduce_max handles 4 chunks per op.
  * Software-pipelined loads: batch b+2's loads issue (sync ring) BEFORE
    batch b's store (scalar ring), so loads never queue behind a 3MB
    store in DMA-ring FIFO order.
  * G block 0 (a verbatim copy of H) never touches the device: the host
    splices the original f32 H into the output during unshard.  The device
    emits [C2Q | H*C2Q | H*Q2C] as one [T,768] f32 block per batch.
  * Tile emits multi-wait instructions; TRN2 allows 1 wait/instruction, so
    the bacc rust passes run on the traced module before compile.
"""

import sys

sys.path.insert(0, "/opt/trn_rl_repo")

import ml_dtypes
import numpy as np

import concourse.bass as bass
import concourse.mybir as mybir
from concourse import tile

B, T, J, D = 64, 1024, 128, 256
NCORES = 8
BPC = B // NCORES  # batches per core
P = 128
NT = T // P  # 8 t-chunks per batch
DA = D + 1  # feature dim + ones column
HT_W = 2 * T  # 2048
IN_W = HT_W + DA  # bf16 blob: [HTb | Ub]
F32 = mybir.dt.float32
BF16 = mybir.dt.bfloat16
AF = mybir.ActivationFunctionType
ALU = mybir.AluOpType
AX = mybir.AxisListType
BF = ml_dtypes.bfloat16


def build_kernel(nc, bpc):
    HF = nc.declare_dram_parameter("hf", [bpc, P, NT, DA], F32, isOutput=False)
    IN = nc.declare_dram_parameter("inb", [bpc, P, IN_W], BF16, isOutput=False)
    whT_in = nc.declare_dram_parameter("whT", [P, 2, 2], BF16, isOutput=False)
    w2_in = nc.declare_dram_parameter("w2", [1, 2 * D], F32, isOutput=False)
    ident_in = nc.declare_dram_parameter("ident", [P, P], BF16, isOutput=False)
    ones_in = nc.declare_dram_parameter("ones1", [1, P], F32, isOutput=False)
    G = nc.declare_dram_parameter("G", [bpc, T, 3 * D], F32, isOutput=True)

    with tile.TileContext(nc) as tc:
        with (
            tc.tile_pool(name="const", bufs=1) as const_pool,
            tc.tile_pool(name="in", bufs=3) as in_pool,
            tc.tile_pool(name="hf", bufs=3) as hf_pool,
            tc.tile_pool(name="u", bufs=2) as u_pool,
            tc.tile_pool(name="p", bufs=2) as p_pool,
            tc.tile_pool(name="g", bufs=2) as g_pool,
            tc.tile_pool(name="sm", bufs=2) as sm_pool,
            tc.tile_pool(name="stps", bufs=1, space="PSUM") as st_ps,
            tc.tile_pool(name="ptps", bufs=2, space="PSUM") as pt_ps,
            tc.tile_pool(name="cqps", bufs=2, space="PSUM") as cq_ps,
            tc.tile_pool(name="smps", bufs=2, space="PSUM") as sm_ps,
        ):
            # ---- constants ----
            ident = const_pool.tile([P, P], BF16)
            nc.sync.dma_start(ident[:], ident_in[:])
            ones1 = const_pool.tile([1, P], F32)
            nc.sync.dma_start(ones1[:], ones_in[:])
            whT = const_pool.tile([P, 2, 2], BF16)
            nc.sync.dma_start(whT[:], whT_in[:])
            # broadcast [w_hu; w_u] across partitions via a K=1 ones-matmul
            w2_sb = const_pool.tile([1, 2 * D], F32)
            nc.sync.dma_start(w2_sb[:], w2_in[:])
            wps = sm_ps.tile([P, 2 * D], F32, tag="sm")
            nc.tensor.matmul(wps[:], ones1[:], w2_sb[:], start=True, stop=True)
            wb = const_pool.tile([P, 2 * D], BF16)
            nc.scalar.copy(wb[:], wps[:])
            whu_b = wb[:, 0:D]
            wu_b = wb[:, D : 2 * D]

            def ht(inb, kc, lo, hi):  # HTb slice: [P, hi-lo] of chunk kc
                return inb[:, kc * T + lo : kc * T + hi]

            inb_tiles = {}
            hf_tiles = {}
            S = {}  # cross-stage per-batch tiles

            def load(b):
                inb_tiles[b] = in_pool.tile([P, IN_W], BF16, name="inb")
                nc.sync.dma_start(inb_tiles[b][:], IN[b])
                hf_tiles[b] = hf_pool.tile([P, NT, DA], F32, name="hf")
                nc.sync.dma_start(hf_tiles[b][:], HF[b])

            def head(b):
                """PE-heavy work independent of batch b's softmax chain."""
                inb = inb_tiles[b]
                Uo = inb[:, HT_W:IN_W]
                Uw = u_pool.tile([P, D], BF16, name="Uw")
                nc.gpsimd.tensor_mul(Uw[:], Uo[:, 0:D], whu_b)
                scr = sm_pool.tile([P, D], F32, name="scr")
                su = sm_pool.tile([P, 1], F32, name="su")
                nc.gpsimd.tensor_mul(scr[:], Uo[:, 0:D], wu_b)
                nc.vector.reduce_sum(su[:], scr[:], axis=AX.X)
                uwt_ps = sm_ps.tile([P, 2, P], BF16, tag="sm", name="uwt_ps")
                for kc in range(2):
                    nc.tensor.transpose(
                        uwt_ps[:, kc, :], Uw[:, kc * P : (kc + 1) * P], ident[:]
                    )
                UwT = u_pool.tile([P, 2, P], BF16, name="UwT")
                nc.scalar.copy(UwT[:], uwt_ps[:])

                # shT[t-chunk, c] = HT-chunk.T @ w_h column
                shT_ps = sm_ps.tile([P, NT, 2], F32, tag="sm", name="shT_ps")
                for c in range(NT):
                    for kc in range(2):
                        nc.tensor.matmul(
                            shT_ps[:, c, :],
                            ht(inb, kc, c * P, (c + 1) * P),
                            whT[:, kc, :],
                            start=(kc == 0),
                            stop=(kc == 1),
                        )
                esh = sm_pool.tile([P, NT], F32, name="esh")
                nc.scalar.activation(esh[:], shT_ps[:, :, 0], AF.Exp)

                # similarity matmul: ST[j, t]
                st = st_ps.tile([P, T], F32, tag="st", name="st")
                for th in range(2):
                    for kc in range(2):
                        nc.tensor.matmul(
                            st[:, th * 512 : (th + 1) * 512],
                            UwT[:, kc, :],
                            ht(inb, kc, th * 512, (th + 1) * 512),
                            start=(kc == 0),
                            stop=(kc == 1),
                        )
                S[b] = dict(su=su, esh=esh, st=st)

            def mid(b):
                """Softmax chain: exp, C2Q + normalize + G2, max-transposes."""
                inb = inb_tiles[b]
                Hf = hf_tiles[b]
                Uo = inb[:, HT_W:IN_W]
                su, st = S[b]["su"], S[b].pop("st")
                Pt = p_pool.tile([P, T], BF16, name="Pt")
                nc.scalar.activation(Pt[:], st[:], AF.Exp, bias=su[:], scale=1.0)

                Gt = g_pool.tile([P, NT, 3 * D], F32, name="Gt")
                linv = sm_pool.tile([P, NT], F32, name="linv")
                for c in range(NT):
                    cq = cq_ps.tile([P, DA], F32, tag="cq", name="cq")
                    nc.tensor.matmul(
                        cq[:], Pt[:, c * P : (c + 1) * P], Uo[:], start=True,
                        stop=True,
                    )
                    nc.vector.reciprocal(linv[:, c : c + 1], cq[:, D : D + 1])
                    if c % 4 == 3:
                        nc.vector.tensor_scalar_mul(
                            Gt[:, c, 0:D], cq[:, 0:D], linv[:, c : c + 1]
                        )
                    else:
                        nc.scalar.activation(
                            Gt[:, c, 0:D], cq[:, 0:D], AF.Copy,
                            scale=linv[:, c : c + 1],
                        )

                # G2 = H * C2Q, pure-f32, 4 chunks per op
                for h, eng in ((0, nc.gpsimd), (1, nc.vector)):
                    cs = slice(h * 4, h * 4 + 4)
                    eng.tensor_mul(
                        Gt[:, cs, D : 2 * D], Hf[:, cs, 0:D], Gt[:, cs, 0:D]
                    )

                # max_j P via PE transpose + DVE reduce
                mx = sm_pool.tile([P, NT], F32, name="mx")
                for h in range(2):
                    pt = pt_ps.tile([P, 4, P], BF16, tag="pt", name="pt")
                    for i in range(4):
                        c = h * 4 + i
                        nc.tensor.transpose(
                            pt[:, i, :], Pt[:, c * P : (c + 1) * P], ident[:]
                        )
                    nc.vector.reduce_max(
                        mx[:, h * 4 : (h + 1) * 4].unsqueeze(2), pt[:], axis=AX.X
                    )
                wq = sm_pool.tile([P, NT], F32, name="wq")
                nc.vector.tensor_mul(wq[:], mx[:], S[b].pop("esh"))
                S[b]["wq"] = wq
                S[b]["Gt"] = Gt

            def tail(b):
                """Q2C (needs wq) + G3 + store."""
                Hf = hf_tiles.pop(b)
                inb_tiles.pop(b)
                wq, Gt = S[b]["wq"], S[b]["Gt"]
                del S[b]
                q2cu_ps = sm_ps.tile([P, DA], F32, tag="sm", name="q2cu_ps")
                for c in range(NT):
                    nc.tensor.matmul(
                        q2cu_ps[:],
                        wq[:, c : c + 1].broadcast_to((P, P)),
                        Hf[:, c, :],
                        start=(c == 0),
                        stop=(c == NT - 1),
                    )
                rinb = sm_pool.tile([P, 1], F32, name="rinb")
                nc.vector.reciprocal(rinb[:], q2cu_ps[:, D : D + 1])
                q2cb = sm_pool.tile([P, D], F32, name="q2cb")
                nc.vector.tensor_scalar_mul(q2cb[:], q2cu_ps[:, 0:D], rinb[:])

                # G3 = H * Q2C, pure-f32 per-chunk 2D ops
                for c in range(NT):
                    eng = nc.gpsimd if c % 2 == 0 else nc.vector
                    eng.tensor_mul(
                        Gt[:, c, 2 * D : 3 * D], Hf[:, c, 0:D], q2cb[:]
                    )

                # store [C2Q | H*C2Q | H*Q2C] on the scalar ring
                Gb = G[b].rearrange("(c p) d -> p c d", p=P)
                nc.scalar.dma_start(Gb[:], Gt[:])

            # software pipeline: PE never waits on batch b's softmax chain --
            # batch b+1's independent head runs while DVE finishes wq(b).
            load(0)
            if bpc > 1:
                load(1)
            head(0)
            for b in range(bpc):
                mid(b)
                if b + 2 < bpc:
                    load(b + 2)
                if b + 1 < bpc:
                    head(b + 1)
                tail(b)

    return nc


_NC_CACHE = {}


def get_nc(bpc=BPC):
    key = bpc
    if key not in _NC_CACHE:
        import bass_rust as _bass_rust

        nc = bass.Bass()
        build_kernel(nc, bpc)
        # TRN2 allows at most 1 sync wait per instruction (2 on event
        # semaphores); Tile emits more.  These are the bacc lowering passes
        # that legalize the wait lists.
        _bass_rust.move_matmul_waits_to_ldweights(nc.m)
        _bass_rust.generate_event_semaphores(nc)
        # lower bass_isa subclasses (e.g. EVENT_SEMAPHORE_RANGE_CLEAR) into
        # raw InstISA encodings walrus can emit
        mybir.codegen_inst_isa_subclasses(nc)
        _NC_CACHE[key] = nc
    return _NC_CACHE[key]


def _prep_core(Hc, Uc):
    """Host-side packing for one core's batches."""
    bpc = Hc.shape[0]
    hf = np.ones((bpc, P, NT, DA), dtype=np.float32)
    hf[..., :D] = Hc.reshape(bpc, NT, P, D).transpose(0, 2, 1, 3)
    blob = np.empty((bpc, P, IN_W), dtype=BF)
    blob[:, :, :HT_W] = (
        Hc.reshape(bpc, T, 2, P).transpose(0, 3, 2, 1).reshape(bpc, P, HT_W)
    )
    ubv = blob[:, :, HT_W:]
    ubv[..., :D] = Uc
    ubv[..., D] = 1.0
    return hf, blob


def run(inputs, trace=False, **kwargs):
    from concourse.bass_utils import run_bass_kernel_spmd

    nc = get_nc(BPC)
    H = np.asarray(inputs["H"], dtype=np.float32)
    U = np.asarray(inputs["U"], dtype=np.float32)
    w_h = np.asarray(inputs["w_h"], dtype=np.float32)
    whT = np.ascontiguousarray(
        np.repeat(w_h.reshape(2, P).T[:, :, None], 2, axis=2)
    ).astype(BF)
    w2 = np.concatenate(
        [
            np.asarray(inputs["w_hu"], dtype=np.float32),
            np.asarray(inputs["w_u"], dtype=np.float32),
        ]
    ).reshape(1, 2 * D)
    ident = np.eye(P, dtype=BF)
    ones1 = np.ones((1, P), dtype=np.float32)
    in_maps = []
    for c in range(NCORES):
        hf, blob = _prep_core(
            H[c * BPC : (c + 1) * BPC], U[c * BPC : (c + 1) * BPC]
        )
        in_maps.append(
            {
                "hf": hf,
                "inb": blob,
                "whT": whT,
                "w2": w2,
                "ident": ident,
                "ones1": ones1,
            }
        )
    res = run_bass_kernel_spmd(
        nc, in_maps, core_ids=list(range(NCORES)), trace=trace, **kwargs
    )
    out = np.empty((B, T, 4 * D), dtype=np.float32)
    out[:, :, 0:D] = H  # G block 0 is a verbatim copy of H
    out[:, :, D:] = np.concatenate(
        [res.results[c]["G"] for c in range(NCORES)], axis=0
    )
    return out, res


def kernel(**inputs):
    out, _ = run(inputs, trace=False)
    return out
